# revision 20
# baseline (speedup 1.0000x reference)
"""Trainium2 Bass kernel for CLIP-style contrastive loss.

loss = 0.5 * (mean_i(lse_row_i - diag_i) + mean_j(lse_col_j - diag_j))
where logits = logit_scale * img @ txt.T, N=16384, D=512.

Strategy (8 cores, no collectives):
  Host transposes both matrices to [D, N] (scale folded into img side).
  Each core runs two symmetric streams:
    stream a: its 2048 img rows x all 16384 txt cols  -> row-lse partials
    stream b: its 2048 txt rows x all 16384 img cols  -> col-lse partials
  Each [128 x 1024] logits supertile (PSUM, 4-deep pipeline) is reduced
  on-chip: DVE reduce_max (negated) -> ACT Exp(bias=-max) with fused
  accum_out row-sum. Per-supertile (negmax, sum) pairs are shipped to the
  host, which does an exact logsumexp combine, adds the diagonal term
  (computed exactly on host), and averages.

  Matmuls use dtype float32r: identical bytes/numerics to fp32 here, but
  streams at 1 cycle/row on the PE instead of fp32's 4 (measured exact,
  ~932us HW). Switching MM_DT to "float8e4" uses fp8 DoubleRow matmuls:
  ~616us HW but ~9e-4 relative error on the final loss.
"""

import numpy as np

# ---- problem constants (hardcoded per harness contract) ----
N = 16384
D = 512
N_CORES = 8
P = 128  # partitions
SUPER_W = 1024  # psum supertile width (2 banks; 4-deep PSUM pipeline)
MM_N = 512  # fp32 moving-operand max free dim

MM_DT = "float32r"  # "float8e4" -> fp8 DoubleRow: 1.5x faster, ~9e-4 rel err

_compiled = {}


def _build(n=N, d=D, n_cores=N_CORES, super_w=SUPER_W, reps=1, mm_dt="float32r",
           rhs_bufs=None, scr_bufs=2):
    import concourse.bass as bass  # noqa: F401
    import concourse.mybir as mybir
    import concourse.tile as tile
    from concourse import bacc
    from contextlib import ExitStack

    F32 = mybir.dt.float32
    MDT = getattr(mybir.dt, mm_dt)
    is_fp8 = mm_dt in ("float8e4", "float8e5")
    HALVES = 2 if is_fp8 else 1  # DoubleRow packs 2 K-rows per partition
    KR = HALVES * P  # contraction rows consumed per matmul
    R = n // n_cores  # own rows per core
    KT = d // KR  # k tiles (matmuls per psum accumulation)
    MC = R // P  # m chunks per core
    NS = n // super_w  # supertiles across full width
    SUB = super_w // MM_N  # 512-wide sub-tiles per supertile
    ST_COLS = MC * NS  # stats columns per stream
    DR = mybir.MatmulPerfMode.DoubleRow if is_fp8 else None

    nc = bacc.Bacc(
        "TRN2", target_bir_lowering=False, debug=False, num_devices=n_cores
    )

    own_a = nc.dram_tensor("own_a", [d, R], MDT, kind="ExternalInput").ap()
    own_b = nc.dram_tensor("own_b", [d, R], MDT, kind="ExternalInput").ap()
    full_a = nc.dram_tensor("full_a", [d, n], MDT, kind="ExternalInput").ap()
    full_b = nc.dram_tensor("full_b", [d, n], MDT, kind="ExternalInput").ap()
    nm_a = nc.dram_tensor("nm_a", [P, ST_COLS], F32, kind="ExternalOutput").ap()
    s_a = nc.dram_tensor("s_a", [P, ST_COLS], F32, kind="ExternalOutput").ap()
    nm_b = nc.dram_tensor("nm_b", [P, ST_COLS], F32, kind="ExternalOutput").ap()
    s_b = nc.dram_tensor("s_b", [P, ST_COLS], F32, kind="ExternalOutput").ap()

    EXP = mybir.ActivationFunctionType.Exp
    AX = mybir.AxisListType.X

    with tile.TileContext(nc) as tc, ExitStack() as ctx:
        if rhs_bufs is None:
            rhs_bufs = 2 * KT
        own_pool = ctx.enter_context(tc.tile_pool(name="own", bufs=2 * KT))
        rhs_pool = ctx.enter_context(tc.tile_pool(name="rhs", bufs=rhs_bufs))
        scr_pool = ctx.enter_context(tc.tile_pool(name="scr", bufs=scr_bufs))
        st_pool = ctx.enter_context(tc.tile_pool(name="st", bufs=2))
        ps_bufs = 4096 // super_w  # 8 PSUM banks = 4096 fp32/partition
        ps_pool = ctx.enter_context(
            tc.tile_pool(name="ps", bufs=ps_bufs, space="PSUM")
        )

        streams = [(own_a, full_b, nm_a, s_a), (own_b, full_a, nm_b, s_b)]
        streams = [(r, *s) for r in range(reps) for s in streams]
        for si, (rep, own_dram, rhs_dram, nm_out, s_out) in enumerate(streams):
            own_tiles = []
            for k in range(KT):
                ot = own_pool.tile([P, HALVES, R], MDT, name="own_t", tag="own_t")
                for h in range(HALVES):
                    r0 = (k * HALVES + h) * P
                    nc.sync.dma_start(ot[:, h, :], own_dram[r0 : r0 + P, :])
                own_tiles.append(ot)
            nm_st = st_pool.tile(
                [P, ST_COLS], F32, name=f"nm_st{si}", tag=f"nm_st{si % 2}"
            )
            s_st = st_pool.tile(
                [P, ST_COLS], F32, name=f"s_st{si}", tag=f"s_st{si % 2}"
            )
            for ci in range(NS):
                rhs_tiles = []
                for k in range(KT):
                    rt = rhs_pool.tile(
                        [P, HALVES, super_w], MDT, name="rhs_t", tag="rhs_t"
                    )
                    for h in range(HALVES):
                        r0 = (k * HALVES + h) * P
                        nc.sync.dma_start(
                            rt[:, h, :],
                            rhs_dram[
                                r0 : r0 + P,
                                ci * super_w : (ci + 1) * super_w,
                            ],
                        )
                    rhs_tiles.append(rt)
                for m in range(MC):
                    ps = ps_pool.tile([P, super_w], F32, name="ps", tag="ps")
                    for k in range(KT):
                        for c in range(SUB):
                            nc.tensor.matmul(
                                ps[:, c * MM_N : (c + 1) * MM_N],
                                lhsT=own_tiles[k][:, :, m * P : (m + 1) * P],
                                rhs=rhs_tiles[k][:, :, c * MM_N : (c + 1) * MM_N],
                                start=(k == 0),
                                stop=(k == KT - 1),
                                perf_mode=DR,
                            )
                    idx = m * NS + ci
                    nc.vector.reduce_max(
                        nm_st[:, idx : idx + 1], ps[:], axis=AX, negate=True
                    )
                    scr = scr_pool.tile([P, super_w], F32, name="scr", tag="scr")
                    nc.scalar.activation(
                        scr[:],
                        ps[:],
                        EXP,
                        bias=nm_st[:, idx : idx + 1],
                        scale=1.0,
                        accum_out=s_st[:, idx : idx + 1],
                    )
            nc.sync.dma_start(nm_out[:], nm_st[:])
            nc.sync.dma_start(s_out[:], s_st[:])

    nc.compile()
    return nc


def _get_nc(key, **kw):
    if key not in _compiled:
        _compiled[key] = _build(**kw)
    return _compiled[key]


def _run_device(A, B, n, n_cores, super_w, trace=False, mm_dt="float32r"):
    """A, B: [d, n] f32 contiguous (A carries the logit scale).

    Returns the bass results (per-core dicts of negmax/sum stats arrays).
    """
    from concourse.bass_utils import run_bass_kernel_spmd

    if mm_dt in ("float8e4", "float8e5"):
        import ml_dtypes

        np_dt = {"float8e4": ml_dtypes.float8_e4m3, "float8e5": ml_dtypes.float8_e5m2}[
            mm_dt
        ]
        A = A.astype(np_dt)
        B = B.astype(np_dt)

    d = A.shape[0]
    R = n // n_cores
    nc = _get_nc(
        (n, d, n_cores, super_w, 1, mm_dt),
        n=n,
        d=d,
        n_cores=n_cores,
        super_w=super_w,
        mm_dt=mm_dt,
    )
    in_maps = []
    for p in range(n_cores):
        sl = slice(p * R, (p + 1) * R)
        in_maps.append(
            {
                "own_a": np.ascontiguousarray(A[:, sl]),
                "own_b": np.ascontiguousarray(B[:, sl]),
                "full_a": A,
                "full_b": B,
            }
        )
    res = run_bass_kernel_spmd(nc, in_maps, core_ids=list(range(n_cores)), trace=trace)
    return res


def _lse_from_stats(nm, s, n, n_cores, super_w):
    """nm, s: [n_cores, P, ST_COLS] -> lse [n] (float64)."""
    R = n // n_cores
    MC = R // P
    NS = n // super_w
    nm = nm.astype(np.float64).reshape(n_cores, P, MC, NS)
    s = s.astype(np.float64).reshape(n_cores, P, MC, NS)
    L = -nm + np.log(s)  # per-supertile lse partial
    m = L.max(axis=3, keepdims=True)
    lse = (m[..., 0] + np.log(np.exp(L - m).sum(axis=3)))  # [cores, P, MC]
    # row index = p*R + mchunk*P + partition
    return lse.transpose(0, 2, 1).reshape(n)


def _compute_loss(image_features, text_features, logit_scale, n=N, d=D,
                  n_cores=N_CORES, super_w=SUPER_W, trace=False, mm_dt="float32r"):
    img = np.asarray(image_features, dtype=np.float32)
    txt = np.asarray(text_features, dtype=np.float32)
    scale = np.float32(np.asarray(logit_scale).reshape(()))
    A = np.ascontiguousarray((scale * img).T)  # [d, n]
    B = np.ascontiguousarray(txt.T)  # [d, n]

    res = _run_device(A, B, n, n_cores, super_w, trace=trace, mm_dt=mm_dt)

    nm_a = np.stack([r["nm_a"] for r in res.results])
    s_a = np.stack([r["s_a"] for r in res.results])
    nm_b = np.stack([r["nm_b"] for r in res.results])
    s_b = np.stack([r["s_b"] for r in res.results])

    row_lse = _lse_from_stats(nm_a, s_a, n, n_cores, super_w)
    col_lse = _lse_from_stats(nm_b, s_b, n, n_cores, super_w)

    diag = np.einsum("dn,dn->n", A.astype(np.float64), B.astype(np.float64))
    loss_i = np.mean(row_lse - diag)
    loss_t = np.mean(col_lse - diag)
    loss = 0.5 * (loss_i + loss_t)
    return np.asarray(loss, dtype=np.float32), res


def kernel(image_features, text_features, logit_scale):
    loss, _ = _compute_loss(image_features, text_features, logit_scale, mm_dt=MM_DT)
    return loss


# revision 29
# speedup vs baseline: 1.4328x; 1.4328x over previous
"""Trainium2 Bass kernel for CLIP-style contrastive loss.

loss = 0.5 * (mean_i(lse_row_i - diag_i) + mean_j(lse_col_j - diag_j))
where logits = logit_scale * img @ txt.T, N=16384, D=512.

Strategy (8 cores, no collectives):
  Host transposes both matrices to [D, N] (scale folded into img side).
  Each core runs two symmetric streams:
    stream a: its 2048 img rows x all 16384 txt cols  -> row-lse partials
    stream b: its 2048 txt rows x all 16384 img cols  -> col-lse partials
  Each [128 x 1024] logits supertile (PSUM, 4-deep pipeline) is reduced
  on-chip: DVE reduce_max (negated) -> ACT Exp(bias=-max) with fused
  accum_out row-sum. Per-supertile (negmax, sum) pairs are shipped to the
  host, which does an exact logsumexp combine, adds the diagonal term
  (computed exactly on host), and averages.

  Matmuls use dtype float32r: identical bytes/numerics to fp32 here, but
  streams at 1 cycle/row on the PE instead of fp32's 4 (measured exact,
  ~932us HW). Switching MM_DT to "float8e4" uses fp8 DoubleRow matmuls:
  ~616us HW but ~9e-4 relative error on the final loss.
"""

import numpy as np

# ---- problem constants (hardcoded per harness contract) ----
N = 16384
D = 512
N_CORES = 8
P = 128  # partitions
SUPER_W = 1024  # psum supertile width (2 banks; 4-deep PSUM pipeline)
MM_N = 512  # fp32 moving-operand max free dim

MM_DT = "float32r"  # "float8e4" -> fp8 DoubleRow: 1.5x faster, ~9e-4 rel err

_compiled = {}


def _build(n=N, d=D, n_cores=N_CORES, super_w=SUPER_W, reps=1, mm_dt="float32r",
           rhs_bufs=None, scr_bufs=2):
    import concourse.bass as bass  # noqa: F401
    import concourse.mybir as mybir
    import concourse.tile as tile
    from concourse import bacc
    from contextlib import ExitStack

    F32 = mybir.dt.float32
    MDT = getattr(mybir.dt, mm_dt)
    is_fp8 = mm_dt in ("float8e4", "float8e5")
    HALVES = 2 if is_fp8 else 1  # DoubleRow packs 2 K-rows per partition
    KR = HALVES * P  # contraction rows consumed per matmul
    R = n // n_cores  # own rows per core
    KT = d // KR  # k tiles (matmuls per psum accumulation)
    MC = R // P  # m chunks per core
    NS = n // super_w  # supertiles across full width
    SUB = super_w // MM_N  # 512-wide sub-tiles per supertile
    ST_COLS = MC * NS  # stats columns per stream
    DR = mybir.MatmulPerfMode.DoubleRow if is_fp8 else None

    nc = bacc.Bacc(
        "TRN2", target_bir_lowering=False, debug=False, num_devices=n_cores
    )

    own_a = nc.dram_tensor("own_a", [d, R], MDT, kind="ExternalInput").ap()
    own_b = nc.dram_tensor("own_b", [d, R], MDT, kind="ExternalInput").ap()
    full_a = nc.dram_tensor("full_a", [d, n], MDT, kind="ExternalInput").ap()
    full_b = nc.dram_tensor("full_b", [d, n], MDT, kind="ExternalInput").ap()
    nm_a = nc.dram_tensor("nm_a", [P, ST_COLS], F32, kind="ExternalOutput").ap()
    s_a = nc.dram_tensor("s_a", [P, ST_COLS], F32, kind="ExternalOutput").ap()
    nm_b = nc.dram_tensor("nm_b", [P, ST_COLS], F32, kind="ExternalOutput").ap()
    s_b = nc.dram_tensor("s_b", [P, ST_COLS], F32, kind="ExternalOutput").ap()

    EXP = mybir.ActivationFunctionType.Exp
    AX = mybir.AxisListType.X

    with tile.TileContext(nc) as tc, ExitStack() as ctx:
        if rhs_bufs is None:
            rhs_bufs = 2 * KT
        own_pool = ctx.enter_context(tc.tile_pool(name="own", bufs=2 * KT))
        rhs_pool = ctx.enter_context(tc.tile_pool(name="rhs", bufs=rhs_bufs))
        scr_pool = ctx.enter_context(tc.tile_pool(name="scr", bufs=scr_bufs))
        st_pool = ctx.enter_context(tc.tile_pool(name="st", bufs=2))
        ps_bufs = 4096 // super_w  # 8 PSUM banks = 4096 fp32/partition
        ps_pool = ctx.enter_context(
            tc.tile_pool(name="ps", bufs=ps_bufs, space="PSUM")
        )

        streams = [(own_a, full_b, nm_a, s_a), (own_b, full_a, nm_b, s_b)]
        streams = [(r, *s) for r in range(reps) for s in streams]
        for si, (rep, own_dram, rhs_dram, nm_out, s_out) in enumerate(streams):
            own_tiles = []
            for k in range(KT):
                ot = own_pool.tile([P, HALVES, R], MDT, name="own_t", tag="own_t")
                for h in range(HALVES):
                    r0 = (k * HALVES + h) * P
                    nc.sync.dma_start(ot[:, h, :], own_dram[r0 : r0 + P, :])
                own_tiles.append(ot)
            nm_st = st_pool.tile(
                [P, ST_COLS], F32, name=f"nm_st{si}", tag=f"nm_st{si % 2}"
            )
            s_st = st_pool.tile(
                [P, ST_COLS], F32, name=f"s_st{si}", tag=f"s_st{si % 2}"
            )
            for ci in range(NS):
                rhs_tiles = []
                for k in range(KT):
                    rt = rhs_pool.tile(
                        [P, HALVES, super_w], MDT, name="rhs_t", tag="rhs_t"
                    )
                    for h in range(HALVES):
                        r0 = (k * HALVES + h) * P
                        nc.sync.dma_start(
                            rt[:, h, :],
                            rhs_dram[
                                r0 : r0 + P,
                                ci * super_w : (ci + 1) * super_w,
                            ],
                        )
                    rhs_tiles.append(rt)
                for m in range(MC):
                    ps = ps_pool.tile([P, super_w], F32, name="ps", tag="ps")
                    for k in range(KT):
                        for c in range(SUB):
                            nc.tensor.matmul(
                                ps[:, c * MM_N : (c + 1) * MM_N],
                                lhsT=own_tiles[k][:, :, m * P : (m + 1) * P],
                                rhs=rhs_tiles[k][:, :, c * MM_N : (c + 1) * MM_N],
                                start=(k == 0),
                                stop=(k == KT - 1),
                                perf_mode=DR,
                            )
                    idx = m * NS + ci
                    nc.vector.reduce_max(
                        nm_st[:, idx : idx + 1], ps[:], axis=AX, negate=True
                    )
                    scr = scr_pool.tile([P, super_w], F32, name="scr", tag="scr")
                    nc.scalar.activation(
                        scr[:],
                        ps[:],
                        EXP,
                        bias=nm_st[:, idx : idx + 1],
                        scale=1.0,
                        accum_out=s_st[:, idx : idx + 1],
                    )
            nc.sync.dma_start(nm_out[:], nm_st[:])
            nc.sync.dma_start(s_out[:], s_st[:])

    nc.compile()
    return nc


def _get_nc(key, **kw):
    if key not in _compiled:
        _compiled[key] = _build(**kw)
    return _compiled[key]


def _run_device(A, B, n, n_cores, super_w, trace=False, mm_dt="float32r"):
    """A, B: [d, n] f32 contiguous (A carries the logit scale).

    Returns the bass results (per-core dicts of negmax/sum stats arrays).
    """
    from concourse.bass_utils import run_bass_kernel_spmd

    if mm_dt in ("float8e4", "float8e5"):
        import ml_dtypes

        np_dt = {"float8e4": ml_dtypes.float8_e4m3, "float8e5": ml_dtypes.float8_e5m2}[
            mm_dt
        ]
        A = A.astype(np_dt)
        B = B.astype(np_dt)

    d = A.shape[0]
    R = n // n_cores
    nc = _get_nc(
        (n, d, n_cores, super_w, 1, mm_dt),
        n=n,
        d=d,
        n_cores=n_cores,
        super_w=super_w,
        mm_dt=mm_dt,
    )
    in_maps = []
    for p in range(n_cores):
        sl = slice(p * R, (p + 1) * R)
        in_maps.append(
            {
                "own_a": np.ascontiguousarray(A[:, sl]),
                "own_b": np.ascontiguousarray(B[:, sl]),
                "full_a": A,
                "full_b": B,
            }
        )
    res = run_bass_kernel_spmd(nc, in_maps, core_ids=list(range(n_cores)), trace=trace)
    return res


def _lse_from_stats(nm, s, n, n_cores, super_w):
    """nm, s: [n_cores, P, ST_COLS] -> lse [n] (float64)."""
    R = n // n_cores
    MC = R // P
    NS = n // super_w
    nm = nm.astype(np.float64).reshape(n_cores, P, MC, NS)
    s = s.astype(np.float64).reshape(n_cores, P, MC, NS)
    L = -nm + np.log(s)  # per-supertile lse partial
    m = L.max(axis=3, keepdims=True)
    lse = (m[..., 0] + np.log(np.exp(L - m).sum(axis=3)))  # [cores, P, MC]
    # row index = p*R + mchunk*P + partition
    return lse.transpose(0, 2, 1).reshape(n)


def _compute_loss(image_features, text_features, logit_scale, n=N, d=D,
                  n_cores=N_CORES, super_w=SUPER_W, trace=False, mm_dt="float32r"):
    img = np.asarray(image_features, dtype=np.float32)
    txt = np.asarray(text_features, dtype=np.float32)
    scale = np.float32(np.asarray(logit_scale).reshape(()))
    A = np.ascontiguousarray((scale * img).T)  # [d, n]
    B = np.ascontiguousarray(txt.T)  # [d, n]

    res = _run_device(A, B, n, n_cores, super_w, trace=trace, mm_dt=mm_dt)

    nm_a = np.stack([r["nm_a"] for r in res.results])
    s_a = np.stack([r["s_a"] for r in res.results])
    nm_b = np.stack([r["nm_b"] for r in res.results])
    s_b = np.stack([r["s_b"] for r in res.results])

    row_lse = _lse_from_stats(nm_a, s_a, n, n_cores, super_w)
    col_lse = _lse_from_stats(nm_b, s_b, n, n_cores, super_w)

    diag = np.einsum("dn,dn->n", A.astype(np.float64), B.astype(np.float64))
    loss_i = np.mean(row_lse - diag)
    loss_t = np.mean(col_lse - diag)
    loss = 0.5 * (loss_i + loss_t)
    return np.asarray(loss, dtype=np.float32), res


def kernel(image_features, text_features, logit_scale):
    loss, _ = _compute_loss(image_features, text_features, logit_scale, mm_dt=MM_DT)
    return loss
